# revision 1
# baseline (speedup 1.0000x reference)
"""Trainium2 Bass kernel for BilinearAttention GNN message passing.

Math (see reference):
  q = (x @ nonneg(q_w).T) / D ; k = (x @ nonneg(k_w).T) / D
  ego = q*q*nonneg(ego_scale)
  G[i,h] = sum_j mask[i,j] * k[src[i,j], h]          (dst grouped per cell)
  sum_local = q * G / (actual_k + 1e-6)
  s = ego + sum_local ; attn = s / (sum_h s + 1e-9)
  res = attn @ nonneg(v_w).T + nonneg(bias)

Distribution: cells sharded over 8 cores. Each core computes q/k for its
cells, AllGathers k into a full table, then gathers neighbor k-rows per
edge with dma_gather (4-row 256B granules; per-edge row is selected from
the granule with predicated copies using host-precomputed one-hot masks).
nonneg(w) = elu(w)+1 = exp(min(w,0)) + max(w,0) computed on device.
"""

import sys

sys.path.insert(0, "/opt/trn_rl_repo")

import numpy as np

import concourse.bacc as bacc
import concourse.bass as bass
import concourse.mybir as mybir
import concourse.tile as tile
from concourse.masks import make_identity

P = 128  # SBUF partitions


class Cfg:
    def __init__(self, N=100000, D=512, H=16, K=32, NC=8, chunk_t=7):
        assert N % NC == 0
        self.N, self.D, self.H, self.K, self.NC = N, D, H, K, NC
        self.n_real = N // NC                    # real cells per core
        self.T = -(-self.n_real // P)            # row tiles per core
        self.C = self.T * P                      # padded cells per core
        self.E = self.C * K                      # padded edges per core
        self.KF = self.C * NC                    # full (padded) table rows
        assert self.KF % 4 == 0
        self.G4 = self.KF // 4                   # 256B granules in table
        assert self.G4 <= 32767, "granule index must fit int16"
        while self.T % chunk_t:
            chunk_t -= 1
        self.CH_T = chunk_t                      # tiles per gather chunk
        self.NCH = self.T // chunk_t             # chunks
        self.SLOT = chunk_t * K                  # edge slots per partition/chunk
        self.EC = self.SLOT * P                  # edges per chunk
        self.DCH = D // P                        # D chunks of 128
        # pad granule: first pad row of shard 0 (zeros)
        self.PADG = self.n_real // 4


def build(cfg: Cfg):
    """Build the Bass program. Returns compiled Bacc object."""
    import os
    STAGE = int(os.environ.get("KSTAGE", "3"))
    SKIP = set(os.environ.get("KSKIP", "").split(","))
    f32, i16 = mybir.dt.float32, mybir.dt.int16
    nc = bacc.Bacc("TRN2", target_bir_lowering=False, debug=False,
                   enable_asserts=False, num_devices=cfg.NC)

    D, H, K, T, C = cfg.D, cfg.H, cfg.K, cfg.T, cfg.C
    QK = 2 * H

    x_sh = nc.dram_tensor("x_sh", [C, D], f32, kind="ExternalInput")
    qkw = nc.dram_tensor("qkw", [QK, D], f32, kind="ExternalInput")
    vw = nc.dram_tensor("vw", [D, H], f32, kind="ExternalInput")
    ego = nc.dram_tensor("ego", [1, H], f32, kind="ExternalInput")
    bias = nc.dram_tensor("bias", [1, D], f32, kind="ExternalInput")
    idxr = nc.dram_tensor("idxr", [cfg.NCH, P, cfg.EC // 16], i16,
                          kind="ExternalInput")
    w4 = nc.dram_tensor("w4", [cfg.NCH, P, cfg.SLOT * 4], mybir.dt.int8,
                        kind="ExternalInput")
    invk = nc.dram_tensor("invk", [T, P], f32, kind="ExternalInput")
    res = nc.dram_tensor("res", [C, D], f32, kind="ExternalOutput")

    ksh = nc.dram_tensor("ksh", [C, H], f32, kind="Internal")
    kfull = nc.dram_tensor("kfull", [cfg.KF, H], f32, kind="Internal",
                           addr_space="Shared")

    with tile.TileContext(nc) as tc:
        with (
            tc.tile_pool(name="const", bufs=1) as cpool,
            tc.tile_pool(name="stream", bufs=3) as spool,
            tc.tile_pool(name="gath", bufs=2) as gpool,
            tc.tile_pool(name="small", bufs=2) as mpool,
            tc.tile_pool(name="ps_big", bufs=2, space="PSUM") as ps_big,
            tc.tile_pool(name="ps_qk", bufs=2, space="PSUM") as ps_qk,
            tc.tile_pool(name="ps_small", bufs=2, space="PSUM") as ps_small,
            tc.tile_pool(name="ps_res", bufs=2, space="PSUM") as ps_res,
        ):
            ident = cpool.tile([P, P], f32)
            make_identity(nc, ident[:])

            # ---- weights prep ----
            # qkw [32, D] -> nonneg -> *1/D -> transposed chunks qkwT [128, DCH, 32]
            qkw_sb = cpool.tile([QK, D], f32)
            nc.sync.dma_start(out=qkw_sb[:], in_=qkw[:, :])
            qkw_nn = cpool.tile([QK, D], f32)
            tmp_e = cpool.tile([QK, D], f32)
            nc.vector.tensor_scalar_min(tmp_e[:], qkw_sb[:], 0.0)
            nc.scalar.activation(tmp_e[:], tmp_e[:],
                                 mybir.ActivationFunctionType.Exp)
            nc.vector.tensor_scalar_max(qkw_nn[:], qkw_sb[:], 0.0)
            nc.vector.tensor_add(qkw_nn[:], qkw_nn[:], tmp_e[:])
            nc.vector.tensor_scalar_mul(qkw_nn[:], qkw_nn[:], 1.0 / D)
            qkwT = cpool.tile([P, cfg.DCH, QK], f32)
            for c in range(cfg.DCH):
                pt = ps_small.tile([P, QK], f32, tag="psm")
                nc.tensor.transpose(out=pt[:], in_=qkw_nn[:, c * P:(c + 1) * P],
                                    identity=ident[:QK, :QK])
                nc.vector.tensor_copy(qkwT[:, c, :], pt[:])

            # vw [D, H] -> vwT [H, D] -> nonneg
            vwT = cpool.tile([H, D], f32)
            for c in range(cfg.DCH):
                vc = mpool.tile([P, H], f32, tag="vld")
                nc.sync.dma_start(out=vc[:], in_=vw[c * P:(c + 1) * P, :])
                pt2 = ps_small.tile([H, P], f32, tag="psm")
                nc.tensor.transpose(out=pt2[:], in_=vc[:], identity=ident[:])
                nc.vector.tensor_copy(vwT[:, c * P:(c + 1) * P], pt2[:])
            vwT_nn = cpool.tile([H, D], f32)
            tmp_v = cpool.tile([H, D], f32)
            nc.vector.tensor_scalar_min(tmp_v[:], vwT[:], 0.0)
            nc.scalar.activation(tmp_v[:], tmp_v[:],
                                 mybir.ActivationFunctionType.Exp)
            nc.vector.tensor_scalar_max(vwT_nn[:], vwT[:], 0.0)
            nc.vector.tensor_add(vwT_nn[:], vwT_nn[:], tmp_v[:])

            # bias [1, D] nonneg
            bias_sb = cpool.tile([1, D], f32)
            nc.sync.dma_start(out=bias_sb[:], in_=bias[:, :])
            bias_nn = cpool.tile([1, D], f32)
            tmp_b = cpool.tile([1, D], f32)
            nc.vector.tensor_scalar_min(tmp_b[:], bias_sb[:], 0.0)
            nc.scalar.activation(tmp_b[:], tmp_b[:],
                                 mybir.ActivationFunctionType.Exp)
            nc.vector.tensor_scalar_max(bias_nn[:], bias_sb[:], 0.0)
            nc.vector.tensor_add(bias_nn[:], bias_nn[:], tmp_b[:])

            # ego [1, H] nonneg -> broadcast to [P, H]
            ego_sb = cpool.tile([1, H], f32)
            nc.sync.dma_start(out=ego_sb[:], in_=ego[:, :])
            ego_nn = cpool.tile([1, H], f32)
            tmp_g = cpool.tile([1, H], f32)
            nc.vector.tensor_scalar_min(tmp_g[:], ego_sb[:], 0.0)
            nc.scalar.activation(tmp_g[:], tmp_g[:],
                                 mybir.ActivationFunctionType.Exp)
            nc.vector.tensor_scalar_max(ego_nn[:], ego_sb[:], 0.0)
            nc.vector.tensor_add(ego_nn[:], ego_nn[:], tmp_g[:])
            ego_bc = cpool.tile([P, H], f32)
            nc.gpsimd.partition_broadcast(ego_bc[:], ego_nn[:])

            ones1 = cpool.tile([1, P], f32)
            nc.vector.memset(ones1[:], 1.0)

            # ---- phase 1: q/k embeddings ----
            qk_all = cpool.tile([P, T, QK], f32)
            for t in range(T):
                x_t = spool.tile([P, D], f32, tag="x")
                nc.sync.dma_start(out=x_t[:], in_=x_sh[t * P:(t + 1) * P, :])
                pxT = ps_big.tile([P, D], f32, tag="pxT")
                for c in range(cfg.DCH):
                    nc.tensor.transpose(out=pxT[:, c * P:(c + 1) * P],
                                        in_=x_t[:, c * P:(c + 1) * P],
                                        identity=ident[:])
                xT = spool.tile([P, D], f32, tag="xT")
                nc.vector.tensor_copy(xT[:], pxT[:])
                pqk = ps_qk.tile([P, QK], f32, tag="pqk")
                for c in range(cfg.DCH):
                    nc.tensor.matmul(pqk[:], lhsT=xT[:, c * P:(c + 1) * P],
                                     rhs=qkwT[:, c, :],
                                     start=(c == 0), stop=(c == cfg.DCH - 1))
                nc.vector.tensor_copy(qk_all[:, t, :], pqk[:])

            if STAGE < 2:
                z = spool.tile([P, D], f32, tag="res")
                nc.vector.memset(z[:], 0.0)
                nc.vector.tensor_copy(z[:, 0:T * QK], qk_all[:].rearrange("p t q -> p (t q)"))
                for t in range(T):
                    nc.sync.dma_start(out=res[t * P:(t + 1) * P, :], in_=z[:])
            # k halves -> ksh (viewed [t, p, h] in DRAM row t*128+p)
            if STAGE >= 2:
                if "ksh" in SKIP:
                    for t in range(T):
                        kx = mpool.tile([P, H], f32, tag="kshw")
                        nc.vector.tensor_copy(kx[:], qk_all[:, t, H:QK])
                        nc.sync.dma_start(out=ksh[t * P:(t + 1) * P, :], in_=kx[:])
                else:
                    ksh_v = ksh[:, :].rearrange("(t p) h -> p t h", p=P)
                    nc.sync.dma_start(out=ksh_v, in_=qk_all[:, :, H:QK])

                # ---- AllGather k table ----
                if "ag" in SKIP:
                    for cc in range(cfg.NC):
                        nc.sync.dma_start(out=kfull[cc * C:(cc + 1) * C, :],
                                          in_=ksh[:, :])
                else:
                    nc.gpsimd.collective_compute(
                        "AllGather", mybir.AluOpType.bypass,
                        replica_groups=[list(range(cfg.NC))],
                        ins=[ksh[:, :].opt()],
                        outs=[kfull[:, :].opt()],
                    )

                # gather source view: [G4, 64]
                ktab = kfull[:, :].rearrange("(g four) h -> g (four h)", four=4)

                # ---- phase 2: edges ----
                for ch in range(cfg.NCH):
                    CH_T, SLOT = cfg.CH_T, cfg.SLOT
                    idx_t = mpool.tile([P, cfg.EC // 16], i16, tag="idx")
                    if "idxl" in SKIP:
                        nc.vector.memset(idx_t[:], 0)
                    else:
                        nc.sync.dma_start(out=idx_t[:], in_=idxr[ch, :, :])
                    w_t = mpool.tile([P, SLOT * 4], mybir.dt.int8, tag="w")
                    if "w" in SKIP:
                        nc.vector.memset(w_t[:], 0)
                    else:
                        nc.sync.dma_start(out=w_t[:], in_=w4[ch, :, :])
                    ik_t = mpool.tile([P, CH_T], f32, tag="ik")
                    if "ik" in SKIP:
                        nc.vector.memset(ik_t[:], 1.0)
                    else:
                        nc.sync.dma_start(
                            out=ik_t[:],
                            in_=invk[ch * CH_T:(ch + 1) * CH_T, :].rearrange("t p -> p t"))

                    kg4 = gpool.tile([P, SLOT, 64], f32, tag="kg4")
                    if "gather" in SKIP:
                        nc.vector.memset(kg4[:], 0.5)
                    else:
                        # SWDGE ring holds ~16K descriptors and one gather's
                        # entry must fit with headroom; split into <=48-slot
                        # (6144-index) sub-gathers.
                        SUB = 48
                        for s0 in range(0, SLOT, SUB):
                            s1 = min(s0 + SUB, SLOT)
                            ni = (s1 - s0) * P
                            nc.gpsimd.dma_gather(
                                kg4[:, s0:s1, :], ktab,
                                idx_t[:, s0 * 8:s1 * 8],
                                num_idxs=ni, num_idxs_reg=ni,
                                elem_size=64, elem_step=64,
                                single_packet=False,
                            )

                    if STAGE < 3:
                        if ch == 0:
                            for tt in range(min(CH_T, 2)):
                                zz = spool.tile([P, D], f32, tag="res")
                                nc.vector.tensor_copy(
                                    zz[:], kg4[:].rearrange("p s e -> p (s e)")[:, tt * D:(tt + 1) * D])
                                nc.sync.dma_start(out=res[tt * P:(tt + 1) * P, :], in_=zz[:])
                        continue
                    # select r via predicated copies: sel[p, s, h]
                    # (inner dim padded to H+1 so APs stay 3D and match the
                    #  broadcast mask shape)
                    sel = gpool.tile([P, SLOT, H + 1], f32, tag="sel", bufs=1)
                    selv = sel[:, :, 0:H]
                    wv = w_t[:].rearrange("p (s r) -> p s r", r=4)
                    nc.vector.memset(sel[:], 0.0)
                    for r in range(0, 4):
                        nc.vector.copy_predicated(
                            out=selv,
                            mask=wv[:, :, r:r + 1].to_broadcast([P, SLOT, H]),
                            data=kg4[:, :, r * H:(r + 1) * H])

                    # G[p, t, h] = sum_j sel[p, (t,j), h]
                    g_t = mpool.tile([P, CH_T, H], f32, tag="g")
                    sel_v = selv.rearrange("p (t j) h -> p t h j", j=K)
                    nc.vector.tensor_reduce(g_t[:], sel_v, axis=mybir.AxisListType.X,
                                            op=mybir.AluOpType.add)

                    q_t = qk_all[:, ch * CH_T:(ch + 1) * CH_T, 0:H]
                    # sum_local = G * q * invk ; ego = q*q*ego_bc ; s = sum
                    ss = mpool.tile([P, CH_T, H], f32, tag="ss")
                    nc.vector.tensor_tensor(out=g_t[:], in0=g_t[:], in1=q_t,
                                            op=mybir.AluOpType.mult)
                    nc.vector.tensor_tensor(
                        out=g_t[:], in0=g_t[:],
                        in1=ik_t[:].unsqueeze(2).to_broadcast([P, CH_T, H]),
                        op=mybir.AluOpType.mult)
                    nc.vector.tensor_tensor(out=ss[:], in0=q_t, in1=q_t,
                                            op=mybir.AluOpType.mult)
                    nc.vector.tensor_tensor(
                        out=ss[:], in0=ss[:],
                        in1=ego_bc[:].unsqueeze(1).to_broadcast([P, CH_T, H]),
                        op=mybir.AluOpType.mult)
                    nc.vector.tensor_add(ss[:], ss[:], g_t[:])
                    # norm
                    nrm = mpool.tile([P, CH_T], f32, tag="nrm")
                    nc.vector.tensor_reduce(nrm[:], ss[:], axis=mybir.AxisListType.X,
                                            op=mybir.AluOpType.add)
                    nc.vector.tensor_scalar_add(nrm[:], nrm[:], 1e-9)
                    nc.vector.reciprocal(nrm[:], nrm[:])
                    nc.vector.tensor_tensor(
                        out=ss[:], in0=ss[:],
                        in1=nrm[:].unsqueeze(2).to_broadcast([P, CH_T, H]),
                        op=mybir.AluOpType.mult)

                    for tt in range(CH_T):
                        psaT = ps_small.tile([H, P], f32, tag="psm")
                        nc.tensor.transpose(out=psaT[:], in_=ss[:, tt, :],
                                            identity=ident[:])
                        saT = mpool.tile([H, P], f32, tag="saT")
                        nc.vector.tensor_copy(saT[:], psaT[:])
                        pres = ps_res.tile([P, D], f32, tag="pres")
                        nc.tensor.matmul(pres[:], lhsT=saT[:], rhs=vwT_nn[:],
                                         start=True, stop=False)
                        nc.tensor.matmul(pres[:], lhsT=ones1[:], rhs=bias_nn[:],
                                         start=False, stop=True)
                        res_sb = spool.tile([P, D], f32, tag="res")
                        nc.vector.tensor_copy(res_sb[:], pres[:])
                        t_glob = ch * CH_T + tt
                        nc.sync.dma_start(out=res[t_glob * P:(t_glob + 1) * P, :],
                                          in_=res_sb[:])

    nc.compile()
    return nc


def prep_inputs(cfg: Cfg, x, adj_list, q_w, k_w, v_w, ego_scale, bias):
    """Host-side sharding/index prep. Returns list of per-core input dicts."""
    N, D, H, K, NC = cfg.N, cfg.D, cfg.H, cfg.K, cfg.NC
    nr, C, T = cfg.n_real, cfg.C, cfg.T
    src = np.asarray(adj_list[0], dtype=np.int64)
    msk = (np.asarray(adj_list[2]) != 0)

    # global row in concatenated padded table
    c0 = src // nr
    loc = c0 * C + (src - c0 * nr)
    gran = loc // 4
    rr = loc % 4
    gran = np.where(msk, gran, cfg.PADG)
    rr = np.where(msk, rr, 0)

    ak = msk.reshape(N, K).sum(axis=1).astype(np.float32)
    inv_ak = (1.0 / (ak + 1e-6)).astype(np.float32)

    qkw_np = np.concatenate([np.asarray(q_w), np.asarray(k_w)], axis=0).astype(np.float32)

    in_maps = []
    for c in range(NC):
        xs = np.zeros((C, D), dtype=np.float32)
        xs[:nr] = np.asarray(x[c * nr:(c + 1) * nr], dtype=np.float32)

        g_c = gran[c * nr * K:(c + 1) * nr * K].reshape(nr, K)
        r_c = rr[c * nr * K:(c + 1) * nr * K].reshape(nr, K)
        m_c = msk[c * nr * K:(c + 1) * nr * K].reshape(nr, K)
        # pad cells
        g_p = np.full((C, K), cfg.PADG, dtype=np.int64)
        g_p[:nr] = g_c
        r_p = np.zeros((C, K), dtype=np.int64)
        r_p[:nr] = r_c
        m_p = np.zeros((C, K), dtype=bool)
        m_p[:nr] = m_c

        ik_p = np.ones((C,), dtype=np.float32)
        ik_p[:nr] = inv_ak[c * nr:(c + 1) * nr]

        # cell (t*128+p), slot s = tt*K + j inside chunk ch (tt in [0,CH_T))
        # gather stream position i = s*128 + p ; idx value = g_p[cell, j]
        cells = np.arange(C).reshape(T, P)          # [t, p]
        idx_np = np.empty((cfg.NCH, P, cfg.EC // 16), dtype=np.int16)
        w_np = np.zeros((cfg.NCH, P, cfg.SLOT, 4), dtype=np.int8)
        for ch in range(cfg.NCH):
            tl = cells[ch * cfg.CH_T:(ch + 1) * cfg.CH_T]   # [CH_T, P]
            # stream [s, p] with s = tt*K+j  -> value g_p[tl[tt,p], j]
            gs = g_p[tl]                                    # [CH_T, P, K]
            stream = gs.transpose(0, 2, 1).reshape(cfg.EC)  # [(tt j) p]
            wrapped = stream.reshape(cfg.EC // 16, 16).T    # [16, EC/16]
            idx_np[ch] = np.broadcast_to(wrapped[None].repeat(8, 0)
                                         .reshape(P, cfg.EC // 16), (P, cfg.EC // 16))
            rs = r_p[tl]                                    # [CH_T, P, K]
            ms = m_p[tl]
            # w[p, s=tt*K+j, r] one-hot (0 if masked)
            s_idx = (np.arange(cfg.CH_T)[:, None] * K + np.arange(K)[None, :])
            for tt in range(cfg.CH_T):
                for j in range(K):
                    s = tt * K + j
                    w_np[ch, :, s, :] = 0.0
                    rsel = rs[tt, :, j]
                    mm = ms[tt, :, j]
                    w_np[ch, np.arange(P), s, rsel] = mm.astype(np.int8)

        in_maps.append({
            "x_sh": xs,
            "qkw": qkw_np,
            "vw": np.asarray(v_w, dtype=np.float32),
            "ego": np.asarray(ego_scale, dtype=np.float32),
            "bias": np.asarray(bias, dtype=np.float32),
            "idxr": idx_np,
            "w4": w_np.reshape(cfg.NCH, P, cfg.SLOT * 4),
            "invk": ik_p.reshape(T, P),
        })
    return in_maps


_CACHE = {}


def _get_compiled(cfg: Cfg):
    key = (cfg.N, cfg.D, cfg.H, cfg.K, cfg.NC)
    if key not in _CACHE:
        _CACHE[key] = build(cfg)
    return _CACHE[key]


def kernel(x, adj_list, q_w, k_w, v_w, ego_scale, bias, _trace=False):
    import concourse.bass_utils as bass_utils
    x = np.asarray(x)
    adj_list = np.asarray(adj_list)
    N, D = x.shape
    H = np.asarray(q_w).shape[0]
    K = adj_list.shape[1] // N
    cfg = Cfg(N=N, D=D, H=H, K=K, NC=8)

    dst = np.asarray(adj_list[1], dtype=np.int64)
    assert np.array_equal(dst, np.repeat(np.arange(N, dtype=np.int64), K)), \
        "kernel requires adj_list[1] grouped per target cell"

    nc = _get_compiled(cfg)
    in_maps = prep_inputs(cfg, x, adj_list, q_w, k_w, v_w, ego_scale, bias)
    r = bass_utils.run_bass_kernel_spmd(nc, in_maps, core_ids=list(range(cfg.NC)),
                                        trace=_trace)
    out = np.concatenate([r.results[c]["res"][:cfg.n_real] for c in range(cfg.NC)],
                         axis=0)
    if _trace:
        return out, r
    return out

